# revision 15
# baseline (speedup 1.0000x reference)
"""Embedding-lookup + row-wise dot kernel for Trainium2 (8 NeuronCores).

Problem (hardcoded, self-contained):
    users:       [16384] int   (values < 1_000_000)
    movies:      [16384] int   (values < 100_000)
    user_table:  [1_000_000, 64] f32
    movie_table: [100_000, 64] f32
    out = sum(user_table[users] * movie_table[movies], axis=-1, keepdims=True)

Strategy (v3 — 4-queue InstDMAGatherAnt, raw Block mode):
  SWDGE Q7 desc-gen (~10ns/desc per queue pair) is the hard bottleneck; the
  ant dma_gather packs ~hundreds of int16 indices per instruction and
  different queue_num (0-3) desc-gen concurrently on different Q7 pairs
  (~4x).  int16 limits one instruction to a 32768-row table window, so the
  user table is vocab-sharded (core c owns windows [4c,4c+4)) with a
  host-side all-to-all on indices, and the movie table is replicated with
  one gather per (user-window x movie-window) cell so rows land directly in
  canonical order.  Raw Block mode with per-window cumulative DMA semaphores
  avoids the Tile framework's small DMA-sem pool, whose reuse waits
  serialize a 32-gather stream; per-window mul+reduce pipelines the DVE
  compute under the gather stream.

  Measured: ~45us HW exec (baseline: 60.2us).  Breakdown: ~7us framework
  prologue, ~9us mlp-library (ant ucode) load stall, ~22-27us gather phase
  (32 instructions, 4-way concurrent desc-gen, ~1us fixed + 10ns/desc per
  instruction on a Q7 pair), ~1us exposed compute tail, ~3us epilogue.
  Known dead ends (measured): InstDMACopy vector-indirect supports exactly
  one index per output partition (128 rows/instr, ~1.1us engine-held,
  single queue only - the baseline's 45us floor); dma_gather num_idxs >
  ~1024 crashes (Q7 idx scratch); all-(-1) idx lists (truncate-to-0)
  crash; multi-index offset APs on InstDMACopy silently misroute unless
  the out AP is exactly 2D [128, inner].
"""

import os
import numpy as np

N_USERS = 1_000_000
N_MOVIES = 100_000
EMB = 64
BATCH = 16384
N_CORES = 8
P = 128
W = 32768
UW_PER_CORE = 4
N_MW = 4
SHARD_ROWS = UW_PER_CORE * W  # 131072

_CACHE = {}


def _ceil(a, b):
    return -(-a // b)


def _plan(users, movies):
    uw = users // W
    core = np.minimum(uw // UW_PER_CORE, N_CORES - 1)
    mw = movies // W

    plans = []
    counts = np.zeros((N_CORES, UW_PER_CORE, N_MW), dtype=np.int64)
    for c in range(N_CORES):
        sel = np.flatnonzero(core == c)
        uwl = uw[sel] - c * UW_PER_CORE
        mwl = mw[sel]
        order = np.lexsort((mwl, uwl))
        elems = sel[order]
        uwl = uwl[order]
        mwl = mwl[order]
        for i in range(UW_PER_CORE):
            for j in range(N_MW):
                counts[c, i, j] = int(np.sum((uwl == i) & (mwl == j)))
        plans.append({"elems": elems, "uwl": uwl, "mwl": mwl})

    cnt_max = counts.max(axis=0)  # [4,4] uniform per-cell instruction size
    # ucode caps one dma_gather at ~1024 indices (4KB Q7 idx scratch)
    assert int(cnt_max.max()) <= 1024, f"cell count {int(cnt_max.max())} > 1024"
    cap = _ceil(np.maximum(cnt_max, 1), 128) * 128  # slots per cell
    return plans, counts, cnt_max, cap


PRE_U = 3  # canonical grid columns prefilled via plain DMA_INDIRECT
PRE_M = 3  # (library-free) during the ~9us mlp-library load stall


def _build_nc(cap, cnt_max):
    import concourse.bacc as bacc
    import concourse.bass as bass
    from concourse import mybir
    from concourse.library_config import mlp

    ncols = cap // 128
    C = int(ncols.sum())
    # idx list layout: per cell, u and m lists both padded to 16 (and the
    # dst capacity to 128); every pad is a valid dummy index 0 (all-(-1)
    # truncation crashes the ucode, and cnt<16 lists are untested territory)
    L16 = [[max(16, _ceil(int(cnt_max[i, j]), 16) * 16) for j in range(N_MW)] for i in range(UW_PER_CORE)]
    idx_cols = sum(sum(r) for r in L16) // 16

    nc = bacc.Bacc(None, target_bir_lowering=False, num_swdge_queues=4)
    ushard_t = nc.dram_tensor("user_shard", [SHARD_ROWS, EMB], mybir.dt.float32, kind="ExternalInput")
    mtable_t = nc.dram_tensor("movie_table", [N_MOVIES, EMB], mybir.dt.float32, kind="ExternalInput")
    uidx_t = nc.dram_tensor("u_idx", [P, idx_cols], mybir.dt.int16, kind="ExternalInput")
    midx_t = nc.dram_tensor("m_idx", [P, idx_cols], mybir.dt.int16, kind="ExternalInput")
    pidx_t = nc.dram_tensor("p_idx", [P, PRE_U + PRE_M], mybir.dt.int32, kind="ExternalInput")
    out_t = nc.dram_tensor("out", [P, C], mybir.dt.float32, kind="ExternalOutput")

    with (
        nc.Block() as block,
        nc.sbuf_tensor("uidx", [P, idx_cols], mybir.dt.int16) as uidx,
        nc.sbuf_tensor("midx", [P, idx_cols], mybir.dt.int16) as midx,
        nc.sbuf_tensor("pidx", [P, PRE_U + PRE_M], mybir.dt.int32) as pidx,
        nc.sbuf_tensor("U", [P, C, EMB], mybir.dt.float32) as U,
        nc.sbuf_tensor("M", [P, C, EMB], mybir.dt.float32) as M,
        nc.sbuf_tensor("prod", [P, C, EMB], mybir.dt.float32) as prod,
        nc.sbuf_tensor("res", [P, C], mybir.dt.float32) as res,
        nc.semaphore("idx_sem") as idx_sem,
        nc.semaphore("gat0") as gat0,
        nc.semaphore("gat1") as gat1,
        nc.semaphore("gat2") as gat2,
        nc.semaphore("gat3") as gat3,
        nc.semaphore("cmp_sem") as cmp_sem,
        nc.semaphore("out_sem") as out_sem,
    ):
        gat_sems = [gat0, gat1, gat2, gat3]
        # gather schedule: (kind, i, j, queue) round-robin across queues,
        # cells interleaved so each queue's desc load is balanced
        sched = []
        rr = 0
        for i in range(UW_PER_CORE):
            for j in range(N_MW):
                sched.append(("u", i, j, rr % 4))
                rr += 1
                sched.append(("m", i, j, rr % 4))
                rr += 1

        colbase = {}
        cb = 0
        off16 = {}
        o = 0
        for i in range(UW_PER_CORE):
            for j in range(N_MW):
                colbase[(i, j)] = cb
                cb += int(ncols[i, j])
                off16[(i, j)] = o
                o += L16[i][j] // 16

        @block.sync
        def _(sync):
            sync.dma_start(out=pidx[:], in_=pidx_t[:]).then_inc(idx_sem, 16)
            sync.dma_start(out=uidx[:], in_=uidx_t[:]).then_inc(idx_sem, 16)
            sync.dma_start(out=midx[:], in_=midx_t[:]).then_inc(idx_sem, 16)
            sync.wait_ge(cmp_sem, UW_PER_CORE)
            sync.dma_start(out=out_t[:], in_=res[:]).then_inc(out_sem, 16)
            sync.wait_ge(out_sem, 16)

        incs = [0] * UW_PER_CORE

        @block.gpsimd
        def _(gpsimd):
            gpsimd.load_library(mlp)
            # prefill the first grid columns with library-free DMA_INDIRECT
            # while the mlp ucode overlay loads (those cols are window 0's)
            gpsimd.wait_ge(idx_sem, 16)
            for k in range(PRE_U):
                gpsimd.indirect_dma_start(
                    out=U[:, k, :],
                    out_offset=None,
                    in_=ushard_t[:],
                    in_offset=bass.IndirectOffsetOnAxis(ap=pidx[:, k : k + 1], axis=0),
                    oob_is_err=False,
                ).then_inc(gat_sems[0], 16)
                incs[0] += 1
            for k in range(PRE_M):
                gpsimd.indirect_dma_start(
                    out=M[:, k, :],
                    out_offset=None,
                    in_=mtable_t[:],
                    in_offset=bass.IndirectOffsetOnAxis(
                        ap=pidx[:, PRE_U + k : PRE_U + k + 1], axis=0
                    ),
                    oob_is_err=False,
                ).then_inc(gat_sems[0], 16)
                incs[0] += 1
            gpsimd.wait_ge(idx_sem, 48)
            for kind, i, j, q in sched:
                n = max(16, int(cnt_max[i, j]))
                dst_col = colbase[(i, j)]
                ncol_ij = int(ncols[i, j])
                o16 = off16[(i, j)]
                pre = PRE_U if kind == "u" else PRE_M
                covered = max(0, min(pre - dst_col, ncol_ij))
                if covered:
                    n -= covered * 128
                    if n <= 0:
                        continue  # cell fully prefilled
                    dst_col += covered
                    ncol_ij -= covered
                    o16 += covered * 8
                nl16 = _ceil(n, 16)
                if kind == "u":
                    gpsimd.dma_gather(
                        out_ap=U[:, dst_col : dst_col + ncol_ij],
                        in_ap=ushard_t[i * W : (i + 1) * W],
                        idxs_ap=uidx[:, o16 : o16 + nl16],
                        num_idxs=n,
                        num_idxs_reg=n,
                        elem_size=EMB,
                        queue_num=q,
                    ).then_inc(gat_sems[i], 16)
                else:
                    ext = min(W, N_MOVIES - j * W)
                    gpsimd.dma_gather(
                        out_ap=M[:, dst_col : dst_col + ncol_ij],
                        in_ap=mtable_t[j * W : j * W + ext],
                        idxs_ap=midx[:, o16 : o16 + nl16],
                        num_idxs=n,
                        num_idxs_reg=n,
                        elem_size=EMB,
                        queue_num=q,
                    ).then_inc(gat_sems[i], 16)
                incs[i] += 1

        @block.vector
        def _(vector):
            # pipelined: window i's mul+reduce runs as soon as its gathers land
            for i in range(UW_PER_CORE):
                c0 = colbase[(i, 0)]
                nc_i = sum(int(ncols[i, j]) for j in range(N_MW))
                vector.wait_ge(gat_sems[i], 16 * incs[i])
                vector.tensor_mul(
                    out=prod[:, c0 : c0 + nc_i], in0=U[:, c0 : c0 + nc_i], in1=M[:, c0 : c0 + nc_i]
                )
                vector.tensor_reduce(
                    out=res[:, c0 : c0 + nc_i],
                    in_=prod[:, c0 : c0 + nc_i],
                    axis=mybir.AxisListType.X,
                    op=mybir.AluOpType.add,
                ).then_inc(cmp_sem, 1)

    nc.compile()
    return nc, C, L16


def _wrap16(flat):
    n = flat.shape[0]
    blk = flat.reshape(n // 16, 16).T
    return np.tile(blk, (8, 1))


def _install_ntff_hook():
    """Shim antenv.axon_hooks (absent in this image) so trace=True works
    under axon, and disable the S3 artifact upload (zero-egress container)."""
    import sys
    import types

    import concourse.bass_utils as bu

    bu.upload_artifacts = lambda d: d

    try:
        from antenv.axon_hooks import get_axon_ntff_profile_hook  # noqa: F401

        return
    except ImportError:
        pass

    import antenv
    from trn_agent_boot.trn_boot import _ntff_profile_via_ctypes

    mod = types.ModuleType("antenv.axon_hooks")
    mod._hook = _ntff_profile_via_ctypes("/opt/axon/libaxon_pjrt.so")
    mod.set_axon_ntff_profile_hook = lambda h: setattr(mod, "_hook", h)
    mod.get_axon_ntff_profile_hook = lambda: mod._hook
    sys.modules["antenv.axon_hooks"] = mod
    antenv.axon_hooks = mod


def kernel(users, movies, user_table, movie_table):
    from concourse.bass_utils import run_bass_kernel_spmd

    users = np.asarray(users).astype(np.int64)
    movies = np.asarray(movies).astype(np.int64)
    user_table = np.ascontiguousarray(np.asarray(user_table, dtype=np.float32))
    movie_table = np.ascontiguousarray(np.asarray(movie_table, dtype=np.float32))

    plans, counts, cnt_max, cap = _plan(users, movies)
    key = (tuple(cap.ravel()), tuple(cnt_max.ravel()))
    if _CACHE.get("key") != key:
        _CACHE["nc"], _CACHE["C"], _CACHE["L16"] = _build_nc(cap, cnt_max)
        _CACHE["key"] = key
    nc, C, L16 = _CACHE["nc"], _CACHE["C"], _CACHE["L16"]

    ncols = cap // 128
    colbase = {}
    cb = 0
    for i in range(UW_PER_CORE):
        for j in range(N_MW):
            colbase[(i, j)] = cb
            cb += int(ncols[i, j])

    in_maps = []
    slot_maps = []
    for c in range(N_CORES):
        pl = plans[c]
        elems, uwl, mwl = pl["elems"], pl["uwl"], pl["mwl"]

        base = c * SHARD_ROWS
        end = min(N_USERS, base + SHARD_ROWS)
        shard = np.zeros((SHARD_ROWS, EMB), dtype=np.float32)
        shard[: end - base] = user_table[base:end]

        C = _CACHE["C"]
        slotfill_u = np.zeros(C * 128, dtype=np.int32)
        slotfill_m = np.zeros(C * 128, dtype=np.int32)
        u_flat = []
        m_flat = []
        batch_ids = []
        slots = []
        for i in range(UW_PER_CORE):
            for j in range(N_MW):
                cell = elems[(uwl == i) & (mwl == j)]
                cnt = cell.shape[0]
                l16 = L16[i][j]
                ulocal = (users[cell] - (c * UW_PER_CORE + i) * W).astype(np.int64)
                ul = np.zeros(l16, dtype=np.int16)
                ul[:cnt] = ulocal.astype(np.int16)
                u_flat.append(ul)
                ml = np.zeros(l16, dtype=np.int16)
                ml[:cnt] = (movies[cell] - j * W).astype(np.int16)
                m_flat.append(ml)
                s0 = colbase[(i, j)] * 128
                slotfill_u[s0 : s0 + cnt] = ulocal + i * W  # shard-local
                slotfill_m[s0 : s0 + cnt] = movies[cell]  # global
                slots.append(s0 + np.arange(cnt))
                batch_ids.append(cell)
        u_flat = np.concatenate(u_flat)
        m_flat = np.concatenate(m_flat)
        slot_maps.append((np.concatenate(batch_ids), np.concatenate(slots)))

        # prefill idx: column k of the grid = slots [k*128, (k+1)*128)
        p_idx = np.empty((128, PRE_U + PRE_M), dtype=np.int32)
        for k in range(PRE_U):
            p_idx[:, k] = slotfill_u[k * 128 : (k + 1) * 128]
        for k in range(PRE_M):
            p_idx[:, PRE_U + k] = slotfill_m[k * 128 : (k + 1) * 128]

        in_maps.append(
            {
                "user_shard": shard,
                "movie_table": movie_table,
                "u_idx": np.ascontiguousarray(_wrap16(u_flat)),
                "m_idx": np.ascontiguousarray(_wrap16(m_flat)),
                "p_idx": np.ascontiguousarray(p_idx),
            }
        )

    trace = os.environ.get("KERNEL_TRACE", "") not in ("", "0")
    if trace:
        try:
            _install_ntff_hook()
        except Exception:
            trace = False
    res = run_bass_kernel_spmd(nc, in_maps, core_ids=list(range(N_CORES)), trace=trace)
    if trace:
        kernel.last_exec_time_ns = res.exec_time_ns
        kernel.last_trace = res.instructions_and_trace

    out = np.zeros((BATCH,), dtype=np.float32)
    for c in range(N_CORES):
        r = res.results[c]["out"]
        batch_ids, slots = slot_maps[c]
        out[batch_ids] = r[slots % 128, slots // 128]
    return out.reshape(BATCH, 1)


# revision 18
# speedup vs baseline: 1.1350x; 1.1350x over previous
"""Embedding-lookup + row-wise dot kernel for Trainium2 (8 NeuronCores).

Problem (hardcoded, self-contained):
    users:       [16384] int   (values < 1_000_000)
    movies:      [16384] int   (values < 100_000)
    user_table:  [1_000_000, 64] f32
    movie_table: [100_000, 64] f32
    out = sum(user_table[users] * movie_table[movies], axis=-1, keepdims=True)

Strategy (v3 — 4-queue InstDMAGatherAnt, raw Block mode):
  SWDGE Q7 desc-gen (~10ns/desc per queue pair) is the hard bottleneck; the
  ant dma_gather packs ~hundreds of int16 indices per instruction and
  different queue_num (0-3) desc-gen concurrently on different Q7 pairs
  (~4x).  int16 limits one instruction to a 32768-row table window, so the
  user table is vocab-sharded (core c owns windows [4c,4c+4)) with a
  host-side all-to-all on indices, and the movie table is replicated with
  one gather per (user-window x movie-window) cell so rows land directly in
  canonical order.  Raw Block mode with per-window cumulative DMA semaphores
  avoids the Tile framework's small DMA-sem pool, whose reuse waits
  serialize a 32-gather stream; per-window mul+reduce pipelines the DVE
  compute under the gather stream.

  Measured: ~45us HW exec (baseline: 60.2us).  Breakdown: ~7us framework
  prologue, ~9us mlp-library (ant ucode) load stall, ~22-27us gather phase
  (32 instructions, 4-way concurrent desc-gen, ~1us fixed + 10ns/desc per
  instruction on a Q7 pair), ~1us exposed compute tail, ~3us epilogue.
  Known dead ends (measured): InstDMACopy vector-indirect supports exactly
  one index per output partition (128 rows/instr, ~1.1us engine-held,
  single queue only - the baseline's 45us floor); dma_gather num_idxs >
  ~1024 crashes (Q7 idx scratch); all-(-1) idx lists (truncate-to-0)
  crash; multi-index offset APs on InstDMACopy silently misroute unless
  the out AP is exactly 2D [128, inner].
"""

import os
import numpy as np

N_USERS = 1_000_000
N_MOVIES = 100_000
EMB = 64
BATCH = 16384
N_CORES = 8
P = 128
W = 32768
UW_PER_CORE = 4
N_MW = 4
SHARD_ROWS = UW_PER_CORE * W  # 131072

_CACHE = {}


def _ceil(a, b):
    return -(-a // b)


def _plan(users, movies):
    uw = users // W
    core = np.minimum(uw // UW_PER_CORE, N_CORES - 1)
    mw = movies // W

    plans = []
    counts = np.zeros((N_CORES, UW_PER_CORE, N_MW), dtype=np.int64)
    for c in range(N_CORES):
        sel = np.flatnonzero(core == c)
        uwl = uw[sel] - c * UW_PER_CORE
        mwl = mw[sel]
        order = np.lexsort((mwl, uwl))
        elems = sel[order]
        uwl = uwl[order]
        mwl = mwl[order]
        for i in range(UW_PER_CORE):
            for j in range(N_MW):
                counts[c, i, j] = int(np.sum((uwl == i) & (mwl == j)))
        plans.append({"elems": elems, "uwl": uwl, "mwl": mwl})

    cnt_max = counts.max(axis=0)  # [4,4] uniform per-cell instruction size
    # ucode caps one dma_gather at ~1024 indices (4KB Q7 idx scratch)
    assert int(cnt_max.max()) <= 1024, f"cell count {int(cnt_max.max())} > 1024"
    cap = _ceil(np.maximum(cnt_max, 1), 128) * 128  # slots per cell
    return plans, counts, cnt_max, cap


def _build_nc(cap, cnt_max):
    import concourse.bacc as bacc
    from concourse import mybir
    from concourse.library_config import mlp

    ncols = cap // 128
    C = int(ncols.sum())
    # idx list layout: per cell, u and m lists both padded to 16 (and the
    # dst capacity to 128); every pad is a valid dummy index 0 (all-(-1)
    # truncation crashes the ucode, and cnt<16 lists are untested territory)
    L16 = [[max(16, _ceil(int(cnt_max[i, j]), 16) * 16) for j in range(N_MW)] for i in range(UW_PER_CORE)]
    idx_cols = sum(sum(r) for r in L16) // 16

    nc = bacc.Bacc(None, target_bir_lowering=False, num_swdge_queues=4)
    ushard_t = nc.dram_tensor("user_shard", [SHARD_ROWS, EMB], mybir.dt.float32, kind="ExternalInput")
    mtable_t = nc.dram_tensor("movie_table", [N_MOVIES, EMB], mybir.dt.float32, kind="ExternalInput")
    uidx_t = nc.dram_tensor("u_idx", [P, idx_cols], mybir.dt.int16, kind="ExternalInput")
    midx_t = nc.dram_tensor("m_idx", [P, idx_cols], mybir.dt.int16, kind="ExternalInput")
    out_t = nc.dram_tensor("out", [P, C], mybir.dt.float32, kind="ExternalOutput")

    n_gather = 2 * UW_PER_CORE * N_MW

    with (
        nc.Block() as block,
        nc.sbuf_tensor("uidx", [P, idx_cols], mybir.dt.int16) as uidx,
        nc.sbuf_tensor("midx", [P, idx_cols], mybir.dt.int16) as midx,
        nc.sbuf_tensor("U", [P, C, EMB], mybir.dt.float32) as U,
        nc.sbuf_tensor("M", [P, C, EMB], mybir.dt.float32) as M,
        nc.sbuf_tensor("prod", [P, C, EMB], mybir.dt.float32) as prod,
        nc.sbuf_tensor("res", [P, C], mybir.dt.float32) as res,
        nc.semaphore("idx_sem") as idx_sem,
        nc.semaphore("gat0") as gat0,
        nc.semaphore("gat1") as gat1,
        nc.semaphore("gat2") as gat2,
        nc.semaphore("gat3") as gat3,
        nc.semaphore("cmp_sem") as cmp_sem,
        nc.semaphore("out_sem") as out_sem,
    ):
        gat_sems = [gat0, gat1, gat2, gat3]
        # gather schedule: (kind, i, j, queue) round-robin across queues,
        # cells interleaved so each queue's desc load is balanced
        sched = []
        rr = 0
        for i in range(UW_PER_CORE):
            for j in range(N_MW):
                sched.append(("u", i, j, rr % 4))
                rr += 1
                sched.append(("m", i, j, rr % 4))
                rr += 1

        colbase = {}
        cb = 0
        off16 = {}
        o = 0
        for i in range(UW_PER_CORE):
            for j in range(N_MW):
                colbase[(i, j)] = cb
                cb += int(ncols[i, j])
                off16[(i, j)] = o
                o += L16[i][j] // 16

        @block.sync
        def _(sync):
            sync.dma_start(out=uidx[:], in_=uidx_t[:]).then_inc(idx_sem, 16)
            sync.dma_start(out=midx[:], in_=midx_t[:]).then_inc(idx_sem, 16)
            sync.wait_ge(cmp_sem, UW_PER_CORE)
            sync.dma_start(out=out_t[:], in_=res[:]).then_inc(out_sem, 16)
            sync.wait_ge(out_sem, 16)

        @block.gpsimd
        def _(gpsimd):
            gpsimd.load_library(mlp)
            gpsimd.wait_ge(idx_sem, 32)
            for kind, i, j, q in sched:
                n = max(16, int(cnt_max[i, j]))
                dst_col = colbase[(i, j)]
                ncol_ij = int(ncols[i, j])
                o16 = off16[(i, j)]
                nl16 = L16[i][j] // 16
                if kind == "u":
                    gpsimd.dma_gather(
                        out_ap=U[:, dst_col : dst_col + ncol_ij],
                        in_ap=ushard_t[i * W : (i + 1) * W],
                        idxs_ap=uidx[:, o16 : o16 + nl16],
                        num_idxs=n,
                        num_idxs_reg=n,
                        elem_size=EMB,
                        queue_num=q,
                    ).then_inc(gat_sems[i], 16)
                else:
                    ext = min(W, N_MOVIES - j * W)
                    gpsimd.dma_gather(
                        out_ap=M[:, dst_col : dst_col + ncol_ij],
                        in_ap=mtable_t[j * W : j * W + ext],
                        idxs_ap=midx[:, o16 : o16 + nl16],
                        num_idxs=n,
                        num_idxs_reg=n,
                        elem_size=EMB,
                        queue_num=q,
                    ).then_inc(gat_sems[i], 16)

        @block.vector
        def _(vector):
            # pipelined: window i's mul+reduce runs as soon as its 8 gathers land
            for i in range(UW_PER_CORE):
                c0 = colbase[(i, 0)]
                nc_i = sum(int(ncols[i, j]) for j in range(N_MW))
                vector.wait_ge(gat_sems[i], 16 * 2 * N_MW)
                vector.tensor_mul(
                    out=prod[:, c0 : c0 + nc_i], in0=U[:, c0 : c0 + nc_i], in1=M[:, c0 : c0 + nc_i]
                )
                vector.tensor_reduce(
                    out=res[:, c0 : c0 + nc_i],
                    in_=prod[:, c0 : c0 + nc_i],
                    axis=mybir.AxisListType.X,
                    op=mybir.AluOpType.add,
                ).then_inc(cmp_sem, 1)

    nc.compile()
    return nc, C, L16


def _wrap16(flat):
    n = flat.shape[0]
    blk = flat.reshape(n // 16, 16).T
    return np.tile(blk, (8, 1))


def _install_ntff_hook():
    """Shim antenv.axon_hooks (absent in this image) so trace=True works
    under axon, and disable the S3 artifact upload (zero-egress container)."""
    import sys
    import types

    import concourse.bass_utils as bu

    bu.upload_artifacts = lambda d: d

    try:
        from antenv.axon_hooks import get_axon_ntff_profile_hook  # noqa: F401

        return
    except ImportError:
        pass

    import antenv
    from trn_agent_boot.trn_boot import _ntff_profile_via_ctypes

    mod = types.ModuleType("antenv.axon_hooks")
    mod._hook = _ntff_profile_via_ctypes("/opt/axon/libaxon_pjrt.so")
    mod.set_axon_ntff_profile_hook = lambda h: setattr(mod, "_hook", h)
    mod.get_axon_ntff_profile_hook = lambda: mod._hook
    sys.modules["antenv.axon_hooks"] = mod
    antenv.axon_hooks = mod


def kernel(users, movies, user_table, movie_table):
    from concourse.bass_utils import run_bass_kernel_spmd

    users = np.asarray(users).astype(np.int64)
    movies = np.asarray(movies).astype(np.int64)
    user_table = np.ascontiguousarray(np.asarray(user_table, dtype=np.float32))
    movie_table = np.ascontiguousarray(np.asarray(movie_table, dtype=np.float32))

    plans, counts, cnt_max, cap = _plan(users, movies)
    key = (tuple(cap.ravel()), tuple(cnt_max.ravel()))
    if _CACHE.get("key") != key:
        _CACHE["nc"], _CACHE["C"], _CACHE["L16"] = _build_nc(cap, cnt_max)
        _CACHE["key"] = key
    nc, C, L16 = _CACHE["nc"], _CACHE["C"], _CACHE["L16"]

    ncols = cap // 128
    colbase = {}
    cb = 0
    for i in range(UW_PER_CORE):
        for j in range(N_MW):
            colbase[(i, j)] = cb
            cb += int(ncols[i, j])

    in_maps = []
    slot_maps = []
    for c in range(N_CORES):
        pl = plans[c]
        elems, uwl, mwl = pl["elems"], pl["uwl"], pl["mwl"]

        base = c * SHARD_ROWS
        end = min(N_USERS, base + SHARD_ROWS)
        shard = np.zeros((SHARD_ROWS, EMB), dtype=np.float32)
        shard[: end - base] = user_table[base:end]

        u_flat = []
        m_flat = []
        batch_ids = []
        slots = []
        for i in range(UW_PER_CORE):
            for j in range(N_MW):
                cell = elems[(uwl == i) & (mwl == j)]
                cnt = cell.shape[0]
                l16 = L16[i][j]
                ul = np.zeros(l16, dtype=np.int16)
                ul[:cnt] = (users[cell] - (c * UW_PER_CORE + i) * W).astype(np.int16)
                u_flat.append(ul)
                ml = np.zeros(l16, dtype=np.int16)
                ml[:cnt] = (movies[cell] - j * W).astype(np.int16)
                m_flat.append(ml)
                s0 = colbase[(i, j)] * 128
                slots.append(s0 + np.arange(cnt))
                batch_ids.append(cell)
        u_flat = np.concatenate(u_flat)
        m_flat = np.concatenate(m_flat)
        slot_maps.append((np.concatenate(batch_ids), np.concatenate(slots)))

        in_maps.append(
            {
                "user_shard": shard,
                "movie_table": movie_table,
                "u_idx": np.ascontiguousarray(_wrap16(u_flat)),
                "m_idx": np.ascontiguousarray(_wrap16(m_flat)),
            }
        )

    trace = os.environ.get("KERNEL_TRACE", "") not in ("", "0")
    if trace:
        try:
            _install_ntff_hook()
        except Exception:
            trace = False
    res = run_bass_kernel_spmd(nc, in_maps, core_ids=list(range(N_CORES)), trace=trace)
    if trace:
        kernel.last_exec_time_ns = res.exec_time_ns
        kernel.last_trace = res.instructions_and_trace

    out = np.zeros((BATCH,), dtype=np.float32)
    for c in range(N_CORES):
        r = res.results[c]["out"]
        batch_ids, slots = slot_maps[c]
        out[batch_ids] = r[slots % 128, slots // 128]
    return out.reshape(BATCH, 1)
